# revision 17
# baseline (speedup 1.0000x reference)
"""Trainium2 8-core kernel for nn_Attn_user_47863115547245.

reference:
    proj     = id_emb @ attn_W.T + attn_b                  # [seq, hid]
    energies = w1*(user @ proj.T) + w2*(socail @ proj.T)   # [state, seq]
    out      = softmax(energies, axis=-1)

Algebraic restructuring (exact up to float rounding):
  * linearity: energies = (w1*user + w2*socail) @ proj.T; the combine
    is folded into host-side input packing (c = ratio*in0 + in1, the
    larger |w| folded into W).
  * reassociation: c @ (W @ id.T) == (c @ W) @ id.T; state(2048) <
    seq(4096) makes (c @ W) first strictly cheaper.
  * attn_b contributes c_i . b, constant along the softmax axis ->
    cancels exactly in softmax; dropped.

Sharding: data-parallel over state rows, 256 rows/core x 8 cores.
id_emb (fp16, pre-transposed) and W (fp16, w-scaled) replicated.
Softmax row-local -> zero collectives.

Schedule (v4; v1 = 65.6us, v2 = 61.3us, v3 = 88us measured):
  * PE streams fp16 matmuls at ~2.35 cols/ns (LDWEIGHTS fully
    overlapped, 216ns per 512-col op). mm1+mm2 = 81920 moving cols
    ~= 35us of PE slots. From mm1's first op the PE should never
    stall; everything else hides behind it.
  * Two HW-DGE rings (sync data from ~8.7us, scalar ~1-2us later).
    Both rings run at roughly HALF rate until ~13us (cross-core HBM
    contention at boot), ~0.21 GB/us each thereafter. W and c are
    packed together per-h ("Wc" pieces, 2.5KB/partition runs, one DMA
    per h) so mm1's first pass gates on a single 0.31MB piece and the
    h-passes track the measured arrival order. id chunks split k-wise
    half/half across rings, arriving 1-3us ahead of consumption.
  * No warmup/garbage matmuls: trace showed full matmul cadence even
    inside HAM k=4 windows.
  * Common-bias softmax: softmax(E) == exp(E-B)/sum(exp(E-B)) exactly
    for any per-row B; only chunk s0 needs a max (B = max_s0 + 40).
    Row-max >= s0-max makes underflow of the anchor impossible; a
    later chunk beating s0 by >128 (~5 sigma) would be needed for
    overflow. pun holds the e^-40-scale exps in bf16. The only tail
    work after the last matmul: exp -> sum -> reciprocal -> per-chunk
    multiply.
  * Tail: chunk order interleaves m1 forward so m0's last chunk ends
    ~3 chunk-slots early; m0's rescale/norm/out-DMA (1MB) then hides
    under m1's remaining matmuls and clears both rings before m1's
    own 1MB needs them. All norms on DVE (ACT must run the final
    exps; gpsimd TENSOR_SCALAR measured ~15us per pair -- 30x slower
    than DVE -- never put compute there). m1's out pairs fan over
    sync/scalar/gpsimd rings; gpsimd's software-DGE ring is slow
    (~0.13 GB/us) but fine for one 0.25MB pair in parallel.
"""

import numpy as np

STATE, SEQ, HID = 2048, 4096, 1024
NCORES = 8
ROWS = STATE // NCORES        # 256 state rows per core
P = 128                       # partitions
KT = HID // P                 # 8 contraction tiles
MT = ROWS // P                # 2 output row tiles per core
SB = 512                      # seq block (one fp32 PSUM bank)
ST = SEQ // SB                # 8 seq blocks
KSYNC = 4                     # id k-slices 0:KSYNC ride sync, rest scalar

# chunk emission order: (s, m) pairs; m1 pulled forward so m0's last
# chunk lands 3 slots before the end and its out-DMA overlaps m1's
# final matmuls. (Pulling m0 earlier than slot 13 makes its out-DMA
# collide with the tail of the id input stream -- measured +2.2us of
# matmul stalls -- so slot 13 it stays.)
CHUNKS = [(0, 0), (0, 1), (1, 0), (1, 1), (2, 0), (2, 1), (3, 0),
          (4, 0), (3, 1), (5, 0), (4, 1), (6, 0), (5, 1), (7, 0),
          (6, 1), (7, 1)]

_graph_cache: list = []


def _build():
    """Build the per-core Bass graph.

    tmpT[k,m] = sum_h W'[h,k] * cT[h,m]                 (mm1, h-outer)
    E[m,s]    = sum_k tmpT[k,m] * idT[k,s]              (mm2)
    out[m,s]  = softmax_s(E)                            (common-bias)
    """
    import concourse.bacc as bacc
    import concourse.mybir as mybir
    import concourse.bass as bass
    from concourse import tile

    f32, f16 = mybir.dt.float32, mybir.dt.float16
    bf16 = mybir.dt.bfloat16
    AX = mybir.AxisListType.X
    ACTF = mybir.ActivationFunctionType

    nc = bacc.Bacc()

    WC = HID + ROWS
    Wc = nc.declare_dram_parameter("Wc", [P, KT, WC], f16, isOutput=False)
    idT = nc.declare_dram_parameter("idT", [ST, P, KT, SB], f16, isOutput=False)
    out = nc.declare_dram_parameter("out", [ROWS, SEQ], f16, isOutput=True)

    with tile.TileContext(nc) as tc:
        with (
            tc.tile_pool(name="sb", bufs=1) as work,
            tc.tile_pool(name="psum", bufs=1,
                         space=bass.MemorySpace.PSUM) as psp,
        ):
            # mm1 accumulator: all 8 banks (one per k group); mm2's
            # rotating chunk accumulators reuse the banks as mm1's
            # copies free them (tile dep tracker serializes the WARs).
            psA = psp.tile([P, KT, SB], f32, tag="acc")

            Wc_sb = work.tile([P, KT, WC], f16)
            id_sb = work.tile([P, ST, KT, SB], f16)

            # ---- input DMAs ----
            for h in range(4):
                nc.sync.dma_start(Wc_sb[:, h, :], Wc[:, h, :])
                nc.scalar.dma_start(Wc_sb[:, h + 4, :], Wc[:, h + 4, :])
            for s in range(ST):
                nc.sync.dma_start(id_sb[:, s, 0:KSYNC, :],
                                  idT[s][:, 0:KSYNC, :])
                nc.scalar.dma_start(id_sb[:, s, KSYNC:KT, :],
                                    idT[s][:, KSYNC:KT, :])

            # ---- mm1: one h-pass into all 8 PSUM banks, h taken in
            # expected DMA arrival order (contraction is commutative).
            HORDER = [0, 4, 1, 5, 2, 6, 3, 7]
            tmpT_sb = work.tile([P, KT, ROWS], f16)
            for hi, h in enumerate(HORDER):
                for kb in range(KT):
                    nc.tensor.matmul(
                        psA[:, kb, :ROWS],
                        Wc_sb[:, h, P * kb:P * (kb + 1)],
                        Wc_sb[:, h, HID:HID + ROWS],
                        start=(hi == 0), stop=(hi == KT - 1),
                    )
            # per-bank copies: mm2's chunk b only waits for copy b
            for kb in range(KT):
                nc.vector.tensor_copy(
                    tmpT_sb[:, kb, :], psA[:, kb, :ROWS])

            # ---- mm2 + common-bias softmax ----
            MARGIN = 40.0
            pun_sb = work.tile([P, MT, SEQ], bf16)
            out_sb = work.tile([P, MT, SEQ], f16)
            Sh = [work.tile([P, ST], f32, tag=f"Sh{m}", name=f"Sh{m}")
                  for m in range(MT)]
            negB = [work.tile([P, 1], f32, tag=f"negB{m}", name=f"negB{m}")
                    for m in range(MT)]
            rinvs = {}

            def rescale(m):
                stot = work.tile([P, 1], f32, tag=f"stot{m}", name=f"stot{m}")
                nc.vector.reduce_sum(stot[:], Sh[m][:], axis=AX)
                rinv = work.tile([P, 1], f32, tag=f"rinv{m}", name=f"rinv{m}")
                nc.vector.reciprocal(rinv[:], stot[:])
                rinvs[m] = rinv

            def norm_pair(m, s, eng):
                lo = SB * s
                if eng is nc.scalar:
                    nc.scalar.activation(
                        out_sb[:, m, lo:lo + 2 * SB],
                        pun_sb[:, m, lo:lo + 2 * SB],
                        ACTF.Copy, scale=rinvs[m][:])
                else:
                    eng.tensor_scalar_mul(
                        out_sb[:, m, lo:lo + 2 * SB],
                        pun_sb[:, m, lo:lo + 2 * SB],
                        rinvs[m][:])

            def out_pair(m, s0, eng):
                eng.dma_start(
                    out[P * m:P * (m + 1), SB * s0:SB * (s0 + 2)],
                    out_sb[:, m, SB * s0:SB * (s0 + 2)])

            for slot, (s, m) in enumerate(CHUNKS):
                ps2 = psA[:, slot % KT, :]
                for k in range(KT):
                    nc.tensor.matmul(
                        ps2[:, :SB],
                        tmpT_sb[:, k, P * m:P * (m + 1)],
                        id_sb[:, s, k, :],
                        start=(k == 0), stop=(k == KT - 1),
                    )
                if s == 0:
                    # the only max: chunk s0 anchors the row bias
                    nmx = work.tile([P, 1], f32, tag=f"nmx{m}",
                                    name=f"nmx{m}")
                    nc.vector.reduce_max(
                        nmx[:], ps2[:, :SB], axis=AX, negate=True)
                    nc.vector.tensor_scalar_add(
                        negB[m][:], nmx[:], -MARGIN)
                nc.scalar.activation(
                    pun_sb[:, m, SB * s:SB * (s + 1)],
                    ps2[:, :SB], ACTF.Exp,
                    bias=negB[m][:], scale=1.0,
                    accum_out=Sh[m][:, s:s + 1],
                )
                if (s, m) == (ST - 1, 0):
                    # m0 complete, 3 chunk-slots early: its whole
                    # normalize + out chain hides under m1's last
                    # matmuls and clears the rings before m1's out.
                    rescale(0)
                    for ss in range(0, ST, 2):
                        norm_pair(0, ss, nc.vector)
                    nc.sync.dma_start(
                        out[0:P, 0:4 * SB], out_sb[:, 0, 0:4 * SB])
                    nc.scalar.dma_start(
                        out[0:P, 4 * SB:SEQ], out_sb[:, 0, 4 * SB:SEQ])

            # m1 tail: gpsimd's slow ring gets the first-normed pair so
            # its data is in flight before the fast rings' second use;
            # ACT (free after the final exp) norms (2-3) in parallel
            # with DVE's three pairs. Issue order front-loads both fast
            # rings: the chip-wide end-of-kernel congestion degrades
            # late transfers, so every issue moved earlier pays off.
            rescale(1)
            norm_pair(1, 0, nc.vector)
            out_pair(1, 0, nc.gpsimd)
            norm_pair(1, 6, nc.vector)
            out_pair(1, 6, nc.sync)
            norm_pair(1, 2, nc.scalar)
            out_pair(1, 2, nc.scalar)
            norm_pair(1, 4, nc.vector)
            out_pair(1, 4, nc.sync)

    nc.compile()
    return nc


def _prepare(user_emb, id_emb, socail_uid_emb, attn_W, w1, w2):
    """Host-side sharding + packing. Returns per-core input maps.

    Packed layouts (per-partition contiguous runs -> efficient DMA
    descriptors):
      Wc:  [128, KT, HID+ROWS] [p,h,:HID] = wbig*W[h*128+p, :],
                               [p,h,HID+m] = c[rows0+m, h*128+p] (fp16)
      idT: [ST, 128, KT, SB]   [s,p,k,c] = id[s*512+c, k*128+p]  (fp16)
    where c = ratio*in0 + in1 (in0 = smaller-|w| side), so that
    wbig * (c @ W @ id.T) equals the reference energies. W and c ride
    in one DMA piece per h: the h-pass gates on a single transfer.
    """
    w1 = float(np.asarray(w1))
    w2 = float(np.asarray(w2))
    swap = abs(w2) > abs(w1)
    wbig = w2 if swap else w1
    wsmall = w1 if swap else w2
    ratio = (wsmall / wbig) if wbig != 0.0 else 0.0

    Wh = (np.float32(wbig) * np.asarray(attn_W, np.float32)).astype(np.float16)
    Wp_pack = np.ascontiguousarray(Wh.reshape(KT, P, HID).transpose(1, 0, 2))

    idh = np.asarray(id_emb, np.float32).astype(np.float16)      # [SEQ, HID]
    idT_pack = np.ascontiguousarray(
        idh.reshape(ST, SB, KT, P).transpose(0, 3, 2, 1)         # [s,p,k,c]
    )

    u = np.asarray(user_emb, np.float32)
    s_ = np.asarray(socail_uid_emb, np.float32)
    in0_full = s_ if not swap else u
    in1_full = u if not swap else s_
    c_full = (np.float32(ratio) * in0_full + in1_full).astype(np.float16)

    in_maps = []
    for i in range(NCORES):
        rows = slice(ROWS * i, ROWS * (i + 1))
        cpk = c_full[rows].reshape(ROWS, KT, P).transpose(2, 1, 0)
        wc = np.concatenate([Wp_pack, cpk], axis=2)   # [P, KT, HID+ROWS]
        in_maps.append({
            "Wc": np.ascontiguousarray(wc),
            "idT": idT_pack,
        })
    return in_maps


def kernel(user_emb, id_emb, socail_uid_emb, attn_W, attn_b, w1, w2):
    from concourse.bass_utils import run_bass_kernel_spmd

    in_maps = _prepare(user_emb, id_emb, socail_uid_emb, attn_W, w1, w2)

    if not _graph_cache:
        _graph_cache.append(_build())
    nc = _graph_cache[0]

    res = run_bass_kernel_spmd(nc, in_maps, core_ids=list(range(NCORES)))
    return np.concatenate(
        [res.results[i]["out"].astype(np.float32) for i in range(NCORES)], axis=0)


# revision 18
# speedup vs baseline: 1.0116x; 1.0116x over previous
"""Trainium2 8-core kernel for nn_Attn_user_47863115547245.

reference:
    proj     = id_emb @ attn_W.T + attn_b                  # [seq, hid]
    energies = w1*(user @ proj.T) + w2*(socail @ proj.T)   # [state, seq]
    out      = softmax(energies, axis=-1)

Algebraic restructuring (exact up to float rounding):
  * linearity: energies = (w1*user + w2*socail) @ proj.T; the combine
    is folded into host-side input packing (c = ratio*in0 + in1, the
    larger |w| folded into W).
  * reassociation: c @ (W @ id.T) == (c @ W) @ id.T; state(2048) <
    seq(4096) makes (c @ W) first strictly cheaper.
  * attn_b contributes c_i . b, constant along the softmax axis ->
    cancels exactly in softmax; dropped.

Sharding: data-parallel over state rows, 256 rows/core x 8 cores.
id_emb (fp16, pre-transposed) and W (fp16, w-scaled) replicated.
Softmax row-local -> zero collectives.

Schedule (final; measured 56.7-58us, baseline 64.7-65.6us.
v1 warmup/2-ring = 65.6; v2 balanced prefix + host combine = 61.3;
v3 gpsimd-norm mistake = 88; v4 Wc-packed prefix = 57.1; final =
v4 + early-finish m0 + fanned tail):
  * PE streams fp16 matmuls at ~2.35 cols/ns (LDWEIGHTS fully
    overlapped, 216ns per 512-col op). mm1+mm2 = 81920 moving cols
    ~= 35us of PE slots. From mm1's first op the PE should never
    stall; everything else hides behind it.
  * Two HW-DGE rings (sync data from ~8.7us, scalar ~1-2us later).
    Both rings run at roughly HALF rate until ~13us (cross-core HBM
    contention at boot), ~0.21 GB/us each thereafter. W and c are
    packed together per-h ("Wc" pieces, 2.5KB/partition runs, one DMA
    per h) so mm1's first pass gates on a single 0.31MB piece and the
    h-passes track the measured arrival order. id chunks split k-wise
    half/half across rings, arriving 1-3us ahead of consumption.
  * No warmup/garbage matmuls: trace showed full matmul cadence even
    inside HAM k=4 windows.
  * Common-bias softmax: softmax(E) == exp(E-B)/sum(exp(E-B)) exactly
    for any per-row B; only chunk s0 needs a max (B = max_s0 + 40).
    Row-max >= s0-max makes underflow of the anchor impossible; a
    later chunk beating s0 by >128 (~5 sigma) would be needed for
    overflow. pun holds the e^-40-scale exps in bf16. The only tail
    work after the last matmul: exp -> sum -> reciprocal -> per-chunk
    multiply.
  * Tail: chunk order interleaves m1 forward so m0's last chunk ends
    ~3 chunk-slots early; m0's rescale/norm/out-DMA (1MB) then hides
    under m1's remaining matmuls and clears both rings before m1's
    own 1MB needs them. All norms on DVE (ACT must run the final
    exps; gpsimd TENSOR_SCALAR measured ~15us per pair -- 30x slower
    than DVE -- never put compute there). m1's out pairs fan over
    sync/scalar/gpsimd rings; gpsimd's software-DGE ring is slow
    (~0.13 GB/us) but fine for one 0.25MB pair in parallel.
"""

import numpy as np

STATE, SEQ, HID = 2048, 4096, 1024
NCORES = 8
ROWS = STATE // NCORES        # 256 state rows per core
P = 128                       # partitions
KT = HID // P                 # 8 contraction tiles
MT = ROWS // P                # 2 output row tiles per core
SB = 512                      # seq block (one fp32 PSUM bank)
ST = SEQ // SB                # 8 seq blocks
KSYNC = 4                     # id k-slices 0:KSYNC ride sync, rest scalar

# chunk emission order: (s, m) pairs; m1 pulled forward so m0's last
# chunk lands 3 slots before the end and its out-DMA overlaps m1's
# final matmuls. (Pulling m0 earlier than slot 13 makes its out-DMA
# collide with the tail of the id input stream -- measured +2.2us of
# matmul stalls -- so slot 13 it stays.)
CHUNKS = [(0, 0), (0, 1), (1, 0), (1, 1), (2, 0), (2, 1), (3, 0),
          (4, 0), (3, 1), (5, 0), (4, 1), (6, 0), (5, 1), (7, 0),
          (6, 1), (7, 1)]

_graph_cache: list = []


def _build():
    """Build the per-core Bass graph.

    tmpT[k,m] = sum_h W'[h,k] * cT[h,m]                 (mm1, h-outer)
    E[m,s]    = sum_k tmpT[k,m] * idT[k,s]              (mm2)
    out[m,s]  = softmax_s(E)                            (common-bias)
    """
    import concourse.bacc as bacc
    import concourse.mybir as mybir
    import concourse.bass as bass
    from concourse import tile

    f32, f16 = mybir.dt.float32, mybir.dt.float16
    bf16 = mybir.dt.bfloat16
    AX = mybir.AxisListType.X
    ACTF = mybir.ActivationFunctionType

    nc = bacc.Bacc()

    WC = HID + ROWS
    Wc = nc.declare_dram_parameter("Wc", [P, KT, WC], f16, isOutput=False)
    idT = nc.declare_dram_parameter("idT", [ST, P, KT, SB], f16, isOutput=False)
    out = nc.declare_dram_parameter("out", [ROWS, SEQ], f16, isOutput=True)

    with tile.TileContext(nc) as tc:
        with (
            tc.tile_pool(name="sb", bufs=1) as work,
            tc.tile_pool(name="psum", bufs=1,
                         space=bass.MemorySpace.PSUM) as psp,
        ):
            # mm1 accumulator: all 8 banks (one per k group); mm2's
            # rotating chunk accumulators reuse the banks as mm1's
            # copies free them (tile dep tracker serializes the WARs).
            psA = psp.tile([P, KT, SB], f32, tag="acc")

            Wc_sb = work.tile([P, KT, WC], f16)
            id_sb = work.tile([P, ST, KT, SB], f16)

            # ---- input DMAs ----
            for h in range(4):
                nc.sync.dma_start(Wc_sb[:, h, :], Wc[:, h, :])
                nc.scalar.dma_start(Wc_sb[:, h + 4, :], Wc[:, h + 4, :])
            for s in range(ST):
                nc.sync.dma_start(id_sb[:, s, 0:KSYNC, :],
                                  idT[s][:, 0:KSYNC, :])
                nc.scalar.dma_start(id_sb[:, s, KSYNC:KT, :],
                                    idT[s][:, KSYNC:KT, :])

            # ---- mm1: one h-pass into all 8 PSUM banks, h taken in
            # expected DMA arrival order (contraction is commutative).
            HORDER = [0, 4, 1, 5, 2, 6, 3, 7]
            tmpT_sb = work.tile([P, KT, ROWS], f16)
            for hi, h in enumerate(HORDER):
                for kb in range(KT):
                    nc.tensor.matmul(
                        psA[:, kb, :ROWS],
                        Wc_sb[:, h, P * kb:P * (kb + 1)],
                        Wc_sb[:, h, HID:HID + ROWS],
                        start=(hi == 0), stop=(hi == KT - 1),
                    )
            # per-bank copies: mm2's chunk b only waits for copy b
            for kb in range(KT):
                nc.vector.tensor_copy(
                    tmpT_sb[:, kb, :], psA[:, kb, :ROWS])

            # ---- mm2 + common-bias softmax ----
            MARGIN = 40.0
            pun_sb = work.tile([P, MT, SEQ], bf16)
            out_sb = work.tile([P, MT, SEQ], f16)
            Sh = [work.tile([P, ST], f32, tag=f"Sh{m}", name=f"Sh{m}")
                  for m in range(MT)]
            negB = [work.tile([P, 1], f32, tag=f"negB{m}", name=f"negB{m}")
                    for m in range(MT)]
            rinvs = {}

            def rescale(m):
                stot = work.tile([P, 1], f32, tag=f"stot{m}", name=f"stot{m}")
                nc.vector.reduce_sum(stot[:], Sh[m][:], axis=AX)
                rinv = work.tile([P, 1], f32, tag=f"rinv{m}", name=f"rinv{m}")
                nc.vector.reciprocal(rinv[:], stot[:])
                rinvs[m] = rinv

            def norm_pair(m, s, eng):
                lo = SB * s
                if eng is nc.scalar:
                    nc.scalar.activation(
                        out_sb[:, m, lo:lo + 2 * SB],
                        pun_sb[:, m, lo:lo + 2 * SB],
                        ACTF.Copy, scale=rinvs[m][:])
                else:
                    eng.tensor_scalar_mul(
                        out_sb[:, m, lo:lo + 2 * SB],
                        pun_sb[:, m, lo:lo + 2 * SB],
                        rinvs[m][:])

            def out_pair(m, s0, eng):
                eng.dma_start(
                    out[P * m:P * (m + 1), SB * s0:SB * (s0 + 2)],
                    out_sb[:, m, SB * s0:SB * (s0 + 2)])

            for slot, (s, m) in enumerate(CHUNKS):
                ps2 = psA[:, slot % KT, :]
                for k in range(KT):
                    nc.tensor.matmul(
                        ps2[:, :SB],
                        tmpT_sb[:, k, P * m:P * (m + 1)],
                        id_sb[:, s, k, :],
                        start=(k == 0), stop=(k == KT - 1),
                    )
                if s == 0:
                    # the only max: chunk s0 anchors the row bias
                    nmx = work.tile([P, 1], f32, tag=f"nmx{m}",
                                    name=f"nmx{m}")
                    nc.vector.reduce_max(
                        nmx[:], ps2[:, :SB], axis=AX, negate=True)
                    nc.vector.tensor_scalar_add(
                        negB[m][:], nmx[:], -MARGIN)
                nc.scalar.activation(
                    pun_sb[:, m, SB * s:SB * (s + 1)],
                    ps2[:, :SB], ACTF.Exp,
                    bias=negB[m][:], scale=1.0,
                    accum_out=Sh[m][:, s:s + 1],
                )
                if (s, m) == (ST - 1, 0):
                    # m0 complete, 3 chunk-slots early: its whole
                    # normalize + out chain hides under m1's last
                    # matmuls and clears the rings before m1's out.
                    rescale(0)
                    for ss in range(0, ST, 2):
                        norm_pair(0, ss, nc.vector)
                    nc.sync.dma_start(
                        out[0:P, 0:4 * SB], out_sb[:, 0, 0:4 * SB])
                    nc.scalar.dma_start(
                        out[0:P, 4 * SB:SEQ], out_sb[:, 0, 4 * SB:SEQ])

            # m1 tail: gpsimd's slow ring gets the first-normed pair so
            # its data is in flight before the fast rings' second use;
            # ACT (free after the final exp) norms (2-3) in parallel
            # with DVE's three pairs. Issue order front-loads both fast
            # rings: the chip-wide end-of-kernel congestion degrades
            # late transfers, so every issue moved earlier pays off.
            rescale(1)
            norm_pair(1, 0, nc.vector)
            out_pair(1, 0, nc.gpsimd)
            norm_pair(1, 6, nc.vector)
            out_pair(1, 6, nc.sync)
            norm_pair(1, 2, nc.scalar)
            out_pair(1, 2, nc.scalar)
            norm_pair(1, 4, nc.vector)
            out_pair(1, 4, nc.sync)

    nc.compile()
    return nc


def _prepare(user_emb, id_emb, socail_uid_emb, attn_W, w1, w2):
    """Host-side sharding + packing. Returns per-core input maps.

    Packed layouts (per-partition contiguous runs -> efficient DMA
    descriptors):
      Wc:  [128, KT, HID+ROWS] [p,h,:HID] = wbig*W[h*128+p, :],
                               [p,h,HID+m] = c[rows0+m, h*128+p] (fp16)
      idT: [ST, 128, KT, SB]   [s,p,k,c] = id[s*512+c, k*128+p]  (fp16)
    where c = ratio*in0 + in1 (in0 = smaller-|w| side), so that
    wbig * (c @ W @ id.T) equals the reference energies. W and c ride
    in one DMA piece per h: the h-pass gates on a single transfer.
    """
    w1 = float(np.asarray(w1))
    w2 = float(np.asarray(w2))
    swap = abs(w2) > abs(w1)
    wbig = w2 if swap else w1
    wsmall = w1 if swap else w2
    ratio = (wsmall / wbig) if wbig != 0.0 else 0.0

    Wh = (np.float32(wbig) * np.asarray(attn_W, np.float32)).astype(np.float16)
    Wp_pack = np.ascontiguousarray(Wh.reshape(KT, P, HID).transpose(1, 0, 2))

    idh = np.asarray(id_emb, np.float32).astype(np.float16)      # [SEQ, HID]
    idT_pack = np.ascontiguousarray(
        idh.reshape(ST, SB, KT, P).transpose(0, 3, 2, 1)         # [s,p,k,c]
    )

    u = np.asarray(user_emb, np.float32)
    s_ = np.asarray(socail_uid_emb, np.float32)
    in0_full = s_ if not swap else u
    in1_full = u if not swap else s_
    c_full = (np.float32(ratio) * in0_full + in1_full).astype(np.float16)

    in_maps = []
    for i in range(NCORES):
        rows = slice(ROWS * i, ROWS * (i + 1))
        cpk = c_full[rows].reshape(ROWS, KT, P).transpose(2, 1, 0)
        wc = np.concatenate([Wp_pack, cpk], axis=2)   # [P, KT, HID+ROWS]
        in_maps.append({
            "Wc": np.ascontiguousarray(wc),
            "idT": idT_pack,
        })
    return in_maps


def kernel(user_emb, id_emb, socail_uid_emb, attn_W, attn_b, w1, w2):
    from concourse.bass_utils import run_bass_kernel_spmd

    in_maps = _prepare(user_emb, id_emb, socail_uid_emb, attn_W, w1, w2)

    if not _graph_cache:
        _graph_cache.append(_build())
    nc = _graph_cache[0]

    res = run_bass_kernel_spmd(nc, in_maps, core_ids=list(range(NCORES)))
    return np.concatenate(
        [res.results[i]["out"].astype(np.float32) for i in range(NCORES)], axis=0)


# revision 19
# speedup vs baseline: 1.0125x; 1.0008x over previous
"""Trainium2 8-core kernel for nn_Attn_user_47863115547245.

reference:
    proj     = id_emb @ attn_W.T + attn_b                  # [seq, hid]
    energies = w1*(user @ proj.T) + w2*(socail @ proj.T)   # [state, seq]
    out      = softmax(energies, axis=-1)

Algebraic restructuring (exact up to float rounding):
  * linearity: energies = (w1*user + w2*socail) @ proj.T; the combine
    is folded into host-side input packing (c = ratio*in0 + in1, the
    larger |w| folded into W).
  * reassociation: c @ (W @ id.T) == (c @ W) @ id.T; state(2048) <
    seq(4096) makes (c @ W) first strictly cheaper.
  * attn_b contributes c_i . b, constant along the softmax axis ->
    cancels exactly in softmax; dropped.

Sharding: data-parallel over state rows, 256 rows/core x 8 cores.
id_emb (fp16, pre-transposed) and W (fp16, w-scaled) replicated.
Softmax row-local -> zero collectives.

Schedule (final; measured 56.7-58us, baseline 64.7-65.6us.
v1 warmup/2-ring = 65.6; v2 balanced prefix + host combine = 61.3;
v3 gpsimd-norm mistake = 88; v4 Wc-packed prefix = 57.1; final =
v4 + early-finish m0 + fanned tail):
  * PE streams fp16 matmuls at ~2.35 cols/ns (LDWEIGHTS fully
    overlapped, 216ns per 512-col op). mm1+mm2 = 81920 moving cols
    ~= 35us of PE slots. From mm1's first op the PE should never
    stall; everything else hides behind it.
  * Two HW-DGE rings (sync data from ~8.7us, scalar ~1-2us later).
    Both rings run at roughly HALF rate until ~13us (cross-core HBM
    contention at boot), ~0.21 GB/us each thereafter. W and c are
    packed together per-h ("Wc" pieces, 2.5KB/partition runs, one DMA
    per h) so mm1's first pass gates on a single 0.31MB piece and the
    h-passes track the measured arrival order. id chunks split k-wise
    half/half across rings, arriving 1-3us ahead of consumption.
  * No warmup/garbage matmuls: trace showed full matmul cadence even
    inside HAM k=4 windows.
  * Common-bias softmax: softmax(E) == exp(E-B)/sum(exp(E-B)) exactly
    for any per-row B; only chunk s0 needs a max (B = max_s0 + 40).
    Row-max >= s0-max makes underflow of the anchor impossible; a
    later chunk beating s0 by >128 (~5 sigma) would be needed for
    overflow. pun holds the e^-40-scale exps in bf16. The only tail
    work after the last matmul: exp -> sum -> reciprocal -> per-chunk
    multiply.
  * Tail: chunk order interleaves m1 forward so m0's last chunk ends
    ~3 chunk-slots early; m0's rescale/norm/out-DMA (1MB) then hides
    under m1's remaining matmuls and clears both rings before m1's
    own 1MB needs them. All norms on DVE (ACT must run the final
    exps; gpsimd TENSOR_SCALAR measured ~15us per pair -- 30x slower
    than DVE -- never put compute there). m1's out pairs fan over
    sync/scalar/gpsimd rings; gpsimd's software-DGE ring is slow
    (~0.13 GB/us) but fine for one 0.25MB pair in parallel.
"""

import numpy as np

STATE, SEQ, HID = 2048, 4096, 1024
NCORES = 8
ROWS = STATE // NCORES        # 256 state rows per core
P = 128                       # partitions
KT = HID // P                 # 8 contraction tiles
MT = ROWS // P                # 2 output row tiles per core
SB = 512                      # seq block (one fp32 PSUM bank)
ST = SEQ // SB                # 8 seq blocks
KSYNC = 4                     # id k-slices 0:KSYNC ride sync, rest scalar

# chunk emission order: (s, m) pairs; m1 pulled forward so m0's last
# chunk lands 3 slots before the end and its out-DMA overlaps m1's
# final matmuls. (Pulling m0 earlier than slot 13 makes its out-DMA
# collide with the tail of the id input stream -- measured +2.2us of
# matmul stalls -- so slot 13 it stays.)
CHUNKS = [(0, 0), (0, 1), (1, 0), (1, 1), (2, 0), (2, 1), (3, 0),
          (4, 0), (3, 1), (5, 0), (4, 1), (6, 0), (5, 1), (7, 0),
          (6, 1), (7, 1)]

_graph_cache: list = []


def _build():
    """Build the per-core Bass graph.

    tmpT[k,m] = sum_h W'[h,k] * cT[h,m]                 (mm1, h-outer)
    E[m,s]    = sum_k tmpT[k,m] * idT[k,s]              (mm2)
    out[m,s]  = softmax_s(E)                            (common-bias)
    """
    import concourse.bacc as bacc
    import concourse.mybir as mybir
    import concourse.bass as bass
    from concourse import tile

    f32, f16 = mybir.dt.float32, mybir.dt.float16
    bf16 = mybir.dt.bfloat16
    AX = mybir.AxisListType.X
    ACTF = mybir.ActivationFunctionType

    nc = bacc.Bacc()

    WC = HID + ROWS
    Wc = nc.declare_dram_parameter("Wc", [P, KT, WC], f16, isOutput=False)
    idT = nc.declare_dram_parameter("idT", [ST, P, KT, SB], f16, isOutput=False)
    out = nc.declare_dram_parameter("out", [ROWS, SEQ], f16, isOutput=True)

    with tile.TileContext(nc) as tc:
        with (
            tc.tile_pool(name="sb", bufs=1) as work,
            tc.tile_pool(name="psum", bufs=1,
                         space=bass.MemorySpace.PSUM) as psp,
        ):
            # mm1 accumulator: all 8 banks (one per k group); mm2's
            # rotating chunk accumulators reuse the banks as mm1's
            # copies free them (tile dep tracker serializes the WARs).
            psA = psp.tile([P, KT, SB], f32, tag="acc")

            Wc_sb = work.tile([P, KT, WC], f16)
            id_sb = work.tile([P, ST, KT, SB], f16)

            # ---- input DMAs ----
            for h in range(4):
                nc.sync.dma_start(Wc_sb[:, h, :], Wc[:, h, :])
                nc.scalar.dma_start(Wc_sb[:, h + 4, :], Wc[:, h + 4, :])
            for s in range(ST):
                nc.sync.dma_start(id_sb[:, s, 0:KSYNC, :],
                                  idT[s][:, 0:KSYNC, :])
                nc.scalar.dma_start(id_sb[:, s, KSYNC:KT, :],
                                    idT[s][:, KSYNC:KT, :])

            # ---- mm1: one h-pass into all 8 PSUM banks, h taken in
            # expected DMA arrival order (contraction is commutative).
            HORDER = [0, 4, 1, 5, 2, 6, 3, 7]
            tmpT_sb = work.tile([P, KT, ROWS], f16)
            for hi, h in enumerate(HORDER):
                for kb in range(KT):
                    nc.tensor.matmul(
                        psA[:, kb, :ROWS],
                        Wc_sb[:, h, P * kb:P * (kb + 1)],
                        Wc_sb[:, h, HID:HID + ROWS],
                        start=(hi == 0), stop=(hi == KT - 1),
                    )
            # per-bank copies: mm2's chunk b only waits for copy b
            for kb in range(KT):
                nc.vector.tensor_copy(
                    tmpT_sb[:, kb, :], psA[:, kb, :ROWS])

            # ---- mm2 + common-bias softmax ----
            MARGIN = 40.0
            pun_sb = work.tile([P, MT, SEQ], bf16)
            out_sb = work.tile([P, MT, SEQ], f16)
            Sh = [work.tile([P, ST], f32, tag=f"Sh{m}", name=f"Sh{m}")
                  for m in range(MT)]
            negB = [work.tile([P, 1], f32, tag=f"negB{m}", name=f"negB{m}")
                    for m in range(MT)]
            rinvs = {}

            def rescale(m):
                stot = work.tile([P, 1], f32, tag=f"stot{m}", name=f"stot{m}")
                nc.vector.reduce_sum(stot[:], Sh[m][:], axis=AX)
                rinv = work.tile([P, 1], f32, tag=f"rinv{m}", name=f"rinv{m}")
                nc.vector.reciprocal(rinv[:], stot[:])
                rinvs[m] = rinv

            def norm_pair(m, s, eng):
                lo = SB * s
                if eng is nc.scalar:
                    nc.scalar.activation(
                        out_sb[:, m, lo:lo + 2 * SB],
                        pun_sb[:, m, lo:lo + 2 * SB],
                        ACTF.Copy, scale=rinvs[m][:])
                else:
                    eng.tensor_scalar_mul(
                        out_sb[:, m, lo:lo + 2 * SB],
                        pun_sb[:, m, lo:lo + 2 * SB],
                        rinvs[m][:])

            def out_pair(m, s0, eng):
                eng.dma_start(
                    out[P * m:P * (m + 1), SB * s0:SB * (s0 + 2)],
                    out_sb[:, m, SB * s0:SB * (s0 + 2)])

            for slot, (s, m) in enumerate(CHUNKS):
                ps2 = psA[:, slot % KT, :]
                for k in range(KT):
                    nc.tensor.matmul(
                        ps2[:, :SB],
                        tmpT_sb[:, k, P * m:P * (m + 1)],
                        id_sb[:, s, k, :],
                        start=(k == 0), stop=(k == KT - 1),
                    )
                if s == 0:
                    # the only max: chunk s0 anchors the row bias
                    nmx = work.tile([P, 1], f32, tag=f"nmx{m}",
                                    name=f"nmx{m}")
                    nc.vector.reduce_max(
                        nmx[:], ps2[:, :SB], axis=AX, negate=True)
                    nc.vector.tensor_scalar_add(
                        negB[m][:], nmx[:], -MARGIN)
                nc.scalar.activation(
                    pun_sb[:, m, SB * s:SB * (s + 1)],
                    ps2[:, :SB], ACTF.Exp,
                    bias=negB[m][:], scale=1.0,
                    accum_out=Sh[m][:, s:s + 1],
                )
                if (s, m) == (ST - 1, 0):
                    # m0 complete, 3 chunk-slots early: its whole
                    # normalize + out chain hides under m1's last
                    # matmuls and clears the rings before m1's out.
                    rescale(0)
                    for ss in range(0, ST, 2):
                        norm_pair(0, ss, nc.vector)
                    nc.sync.dma_start(
                        out[0:P, 0:4 * SB], out_sb[:, 0, 0:4 * SB])
                    nc.scalar.dma_start(
                        out[0:P, 4 * SB:SEQ], out_sb[:, 0, 4 * SB:SEQ])

            # m1 tail: out rides as two 0.5MB quads -- 4KB/partition
            # runs move ~0.36 GB/us vs ~0.17 for 2KB-run pairs
            # (measured). DVE norms three pairs, ACT (free after the
            # final exp) the fourth, so the scalar quad isn't stuck
            # behind DVE's serial chain. No gpsimd DMA anywhere: its
            # teardown drain sat on the end-of-kernel barrier.
            rescale(1)
            norm_pair(1, 0, nc.vector)
            norm_pair(1, 2, nc.vector)
            nc.sync.dma_start(
                out[P:2 * P, 0:4 * SB], out_sb[:, 1, 0:4 * SB])
            norm_pair(1, 4, nc.vector)
            norm_pair(1, 6, nc.scalar)
            nc.scalar.dma_start(
                out[P:2 * P, 4 * SB:SEQ], out_sb[:, 1, 4 * SB:SEQ])

    nc.compile()
    return nc


def _prepare(user_emb, id_emb, socail_uid_emb, attn_W, w1, w2):
    """Host-side sharding + packing. Returns per-core input maps.

    Packed layouts (per-partition contiguous runs -> efficient DMA
    descriptors):
      Wc:  [128, KT, HID+ROWS] [p,h,:HID] = wbig*W[h*128+p, :],
                               [p,h,HID+m] = c[rows0+m, h*128+p] (fp16)
      idT: [ST, 128, KT, SB]   [s,p,k,c] = id[s*512+c, k*128+p]  (fp16)
    where c = ratio*in0 + in1 (in0 = smaller-|w| side), so that
    wbig * (c @ W @ id.T) equals the reference energies. W and c ride
    in one DMA piece per h: the h-pass gates on a single transfer.
    """
    w1 = float(np.asarray(w1))
    w2 = float(np.asarray(w2))
    swap = abs(w2) > abs(w1)
    wbig = w2 if swap else w1
    wsmall = w1 if swap else w2
    ratio = (wsmall / wbig) if wbig != 0.0 else 0.0

    Wh = (np.float32(wbig) * np.asarray(attn_W, np.float32)).astype(np.float16)
    Wp_pack = np.ascontiguousarray(Wh.reshape(KT, P, HID).transpose(1, 0, 2))

    idh = np.asarray(id_emb, np.float32).astype(np.float16)      # [SEQ, HID]
    idT_pack = np.ascontiguousarray(
        idh.reshape(ST, SB, KT, P).transpose(0, 3, 2, 1)         # [s,p,k,c]
    )

    u = np.asarray(user_emb, np.float32)
    s_ = np.asarray(socail_uid_emb, np.float32)
    in0_full = s_ if not swap else u
    in1_full = u if not swap else s_
    c_full = (np.float32(ratio) * in0_full + in1_full).astype(np.float16)

    in_maps = []
    for i in range(NCORES):
        rows = slice(ROWS * i, ROWS * (i + 1))
        cpk = c_full[rows].reshape(ROWS, KT, P).transpose(2, 1, 0)
        wc = np.concatenate([Wp_pack, cpk], axis=2)   # [P, KT, HID+ROWS]
        in_maps.append({
            "Wc": np.ascontiguousarray(wc),
            "idT": idT_pack,
        })
    return in_maps


def kernel(user_emb, id_emb, socail_uid_emb, attn_W, attn_b, w1, w2):
    from concourse.bass_utils import run_bass_kernel_spmd

    in_maps = _prepare(user_emb, id_emb, socail_uid_emb, attn_W, w1, w2)

    if not _graph_cache:
        _graph_cache.append(_build())
    nc = _graph_cache[0]

    res = run_bass_kernel_spmd(nc, in_maps, core_ids=list(range(NCORES)))
    return np.concatenate(
        [res.results[i]["out"].astype(np.float32) for i in range(NCORES)], axis=0)
